# revision 16
# baseline (speedup 1.0000x reference)
"""Bispectrum kernel for Trainium2 (8 NeuronCores, batch-parallel).

Math: per signal x[L] (L=256), the reference computes
    TM[l1,l2] = (1/L) * sum_n x[n] * x[n-l1] * x[n-l2]   (x zero for idx<0)
averaged over T=8 targets.  Substituting p = L-1-n and
z = [reverse(x)*s, zeros(L)] with s = (L*T)^(-1/3):
    mean_t TM[l1,l2] = sum_t sum_p z[p] * z[p+l1] * z[p+l2]
All three factors come from a Hankel matrix H[p,j] = z[p+j], built by a
single contiguous DMA per batch element from host-gathered windows.  On
the PE: out[l1,l2] += lhsT[p,l1] . rhs[p,l2] with lhsT[p,l1] =
z[p]*z[p+l1] (per-partition scale of H) and rhs[p,l2] = z[p+l2] (slices
of H), accumulating over the two 128-row K-chunks and the 8 targets
directly in PSUM.  Structure exploited:
  * K-chunk p in [128,256) has identically-zero lhsT for l1 >= 128 and
    zero rhs columns for l2 >= 128 -> it only feeds the [l1<128, l2<128]
    quadrant (N=128 matmuls).
  * TM is symmetric -> the [l1>=128, l2<128] quadrant is the transpose
    of [l1<128, l2>=128]; computed on host from the other tile, so the
    second row-tile needs only l2 in [128,256) (N=128 matmuls).
Per batch element: 8 N=256 + 16 N=128 fp16 matmuls (1 cyc/row on PE).
The cube-root prescale makes PSUM hold the final averaged values.
"""

import numpy as np

import concourse.bacc as bacc
import concourse.bass as bass
import concourse.mybir as mybir
import concourse.tile as tile
from concourse.bass_utils import run_bass_kernel_spmd

# Problem shape (hardcoded per contract).
B, T, L = 32, 8, 256
N_CORES = 8
B_CORE = B // N_CORES  # 4 batch elements per core
ZLEN = 2 * L           # 512

_F16 = mybir.dt.float16
_F32 = mybir.dt.float32
_COPY = mybir.ActivationFunctionType.Copy


def _build_nc():
    nc = bacc.Bacc("TRN2", target_bir_lowering=False, debug=False)
    # zh[b, p, t, j] = fp16(z[b, t, p+j])  (host-side im2col)
    zh = nc.dram_tensor("zh", [B_CORE, 128, T, 256], _F16, kind="ExternalInput")
    # zc[p, 16*b+8*c+t] = z[b, t, 128*c + p]  (f32 scale columns, padded
    # to 512B/partition so the DMA descriptors hit the fast path)
    zc = nc.dram_tensor("zc", [128, 128], _F32, kind="ExternalInput")
    out = nc.dram_tensor("out", [B_CORE, L, L], _F32, kind="ExternalOutput")

    with tile.TileContext(nc) as tc:
        with (
            tc.tile_pool(name="hank", bufs=1) as hank_pool,
            tc.tile_pool(name="a0", bufs=2) as a0_pool,
            tc.tile_pool(name="a1", bufs=2) as a1_pool,
            tc.tile_pool(name="ps0", bufs=2, space="PSUM") as ps0_pool,
            tc.tile_pool(name="ps1", bufs=2, space="PSUM") as ps1_pool,
            tc.tile_pool(name="osb", bufs=4) as osb_pool,
        ):
            zct = hank_pool.tile([128, 128], _F32, tag="zct")
            nc.scalar.dma_start(out=zct[:], in_=zc[:])
            # PE warm-up: ~3us of junk matmuls while the first hank DMA is
            # in flight, so the HAM clock gate reaches 8/8 before real work.
            warm = hank_pool.tile([128, 256], _F16, tag="warm")
            nc.vector.memset(warm[:], 0.0)
            wps = ps0_pool.tile([128, 256], _F32, tag="warmps")
            for _ in range(12):
                nc.tensor.matmul(
                    wps[:], lhsT=warm[:, 0:128], rhs=warm[:],
                    start=True, stop=True,
                )
            hanks = []
            for b in range(B_CORE):
                hank = hank_pool.tile([128, T, 256], _F16, tag=f"hank{b}")
                if b == 0:
                    nc.sync.dma_start(out=hank[:, 0:4, :], in_=zh[b, :, 0:4, :])
                    nc.sync.dma_start(out=hank[:, 4:8, :], in_=zh[b, :, 4:8, :])
                else:
                    nc.sync.dma_start(out=hank[:], in_=zh[b])
                hanks.append(hank)

            for b in range(B_CORE):
                hank = hanks[b]
                a0 = a0_pool.tile([128, T, 256], _F16)
                a1 = a1_pool.tile([128, T, 128], _F16)
                for t in range(T):
                    # a0[p,t,l] = z[p] * z[p+l]      (DVE)
                    nc.vector.tensor_scalar_mul(
                        a0[:, t, :], hank[:, t, :], zct[:, 16 * b + t:16 * b + t + 1]
                    )
                    # a1[p,t,l] = z[128+p] * z[128+p+l]  (split DVE/ACT)
                    if t % 2 == 0:
                        nc.vector.tensor_scalar_mul(
                            a1[:, t, :], hank[:, t, 128:256],
                            zct[:, 16 * b + 8 + t:16 * b + 8 + t + 1],
                        )
                    else:
                        nc.scalar.activation(
                            a1[:, t, :], hank[:, t, 128:256], _COPY,
                            scale=zct[:, 16 * b + 8 + t:16 * b + 8 + t + 1],
                        )

                # Row-tile m=0: full l2 range; chunk1 feeds only l2 < 128.
                ps0 = ps0_pool.tile([128, 256], _F32)
                for t in range(T):
                    nc.tensor.matmul(
                        ps0[:], lhsT=a0[:, t, 0:128], rhs=hank[:, t, 0:256],
                        start=(t == 0), stop=False,
                    )
                for t in range(T):
                    nc.tensor.matmul(
                        ps0[:, 0:128], lhsT=a1[:, t, :],
                        rhs=hank[:, t, 128:256],
                        start=False, stop=(t == T - 1),
                    )
                # Row-tile m=1: only l2 in [128,256); lower-left quadrant is
                # the transpose of m=0's right half (host fills it).
                ps1 = ps1_pool.tile([128, 128], _F32)
                for t in range(T):
                    nc.tensor.matmul(
                        ps1[:], lhsT=a0[:, t, 128:256],
                        rhs=hank[:, t, 128:256],
                        start=(t == 0), stop=(t == T - 1),
                    )

                osb0 = osb_pool.tile([128, 256], _F32, tag="osb0")
                nc.scalar.activation(osb0[:], ps0[:], _COPY)
                nc.sync.dma_start(out=out[b, 0:128, :], in_=osb0[:])
                osb1 = osb_pool.tile([128, 128], _F32, tag="osb1")
                nc.scalar.activation(osb1[:], ps1[:], _COPY)
                nc.sync.dma_start(out=out[b, 128:256, 128:256], in_=osb1[:])
    nc.finalize()
    return nc


_NC_CACHE = None


def get_nc():
    global _NC_CACHE
    if _NC_CACHE is None:
        _NC_CACHE = _build_nc()
    return _NC_CACHE


def prepare_in_maps(target: np.ndarray):
    """Host prep: reversed/prescaled z, Hankel-window gather, fp16 cast."""
    target = np.ascontiguousarray(np.asarray(target, dtype=np.float32))
    assert target.shape == (B, T, L), target.shape
    s = np.float32((L * T) ** (-1.0 / 3.0))
    z = np.zeros((B, T, ZLEN), np.float32)
    z[:, :, :L] = target[:, :, ::-1] * s
    # win[b, t, p, j] = z[b, t, p+j]
    win = np.lib.stride_tricks.sliding_window_view(z, 256, axis=2)[:, :, :128, :]
    zh = np.ascontiguousarray(win.transpose(0, 2, 1, 3).astype(np.float16))
    # zc[p, 16b+8c+t] = z[b, t, 128c + p], zero-padded to 128 cols
    zcf = np.stack([z[:, :, 0:128], z[:, :, 128:256]], axis=2)  # [B, T, 2, 128]
    zcf = zcf.transpose(3, 0, 2, 1).reshape(128, B, 16)          # [128, B, 2*T]
    in_maps = [
        {
            "zh": np.ascontiguousarray(zh[i * B_CORE:(i + 1) * B_CORE]),
            "zc": np.concatenate(
                [zcf[:, i * B_CORE:(i + 1) * B_CORE].reshape(128, 64),
                 np.zeros((128, 64), np.float32)], axis=1),
        }
        for i in range(N_CORES)
    ]

    def assemble(results):
        source = np.concatenate(
            [results[i]["out"] for i in range(N_CORES)], axis=0
        )
        # lower-left quadrant by symmetry
        source[:, 128:256, 0:128] = source[:, 0:128, 128:256].transpose(0, 2, 1)
        return source[:, None, :, :]

    return in_maps, assemble


def kernel(target: np.ndarray) -> tuple[np.ndarray, np.ndarray]:
    target = np.ascontiguousarray(np.asarray(target, dtype=np.float32))
    in_maps, assemble = prepare_in_maps(target)
    res = run_bass_kernel_spmd(get_nc(), in_maps, list(range(N_CORES)))
    return assemble(res.results), target


# revision 17
# speedup vs baseline: 1.0753x; 1.0753x over previous
"""Bispectrum kernel for Trainium2 (8 NeuronCores, batch-parallel).

Math: per signal x[L] (L=256), the reference computes
    TM[l1,l2] = (1/L) * sum_n x[n] * x[n-l1] * x[n-l2]   (x zero for idx<0)
averaged over T=8 targets.  Substituting p = L-1-n and
z = [reverse(x)*s, zeros(L)] with s = (L*T)^(-1/3):
    mean_t TM[l1,l2] = sum_t sum_p z[p] * z[p+l1] * z[p+l2]
All three factors come from a Hankel matrix H[p,j] = z[p+j], built by a
single contiguous DMA per batch element from host-gathered windows.  On
the PE: out[l1,l2] += lhsT[p,l1] . rhs[p,l2] with lhsT[p,l1] =
z[p]*z[p+l1] (per-partition scale of H) and rhs[p,l2] = z[p+l2] (slices
of H), accumulating over the two 128-row K-chunks and the 8 targets
directly in PSUM.  Structure exploited:
  * K-chunk p in [128,256) has identically-zero lhsT for l1 >= 128 and
    zero rhs columns for l2 >= 128 -> it only feeds the [l1<128, l2<128]
    quadrant (N=128 matmuls).
  * TM is symmetric -> the [l1>=128, l2<128] quadrant is the transpose
    of [l1<128, l2>=128]; computed on host from the other tile, so the
    second row-tile needs only l2 in [128,256) (N=128 matmuls).
Per batch element: 8 N=256 + 16 N=128 fp16 matmuls (1 cyc/row on PE).
The cube-root prescale makes PSUM hold the final averaged values.
"""

import numpy as np

import concourse.bacc as bacc
import concourse.bass as bass
import concourse.mybir as mybir
import concourse.tile as tile
from concourse.bass_utils import run_bass_kernel_spmd

# Problem shape (hardcoded per contract).
B, T, L = 32, 8, 256
N_CORES = 8
B_CORE = B // N_CORES  # 4 batch elements per core
ZLEN = 2 * L           # 512

_F16 = mybir.dt.float16
_F32 = mybir.dt.float32
_COPY = mybir.ActivationFunctionType.Copy


def _build_nc():
    nc = bacc.Bacc("TRN2", target_bir_lowering=False, debug=False)
    # zh[b, p, t, j] = fp16(z[b, t, p+j])  (host-side im2col)
    zh = nc.dram_tensor("zh", [B_CORE, 128, T, 256], _F16, kind="ExternalInput")
    # zc[p, 16*b+8*c+t] = z[b, t, 128*c + p]  (f32 scale columns, padded
    # to 512B/partition so the DMA descriptors hit the fast path)
    zc = nc.dram_tensor("zc", [128, 128], _F32, kind="ExternalInput")
    out = nc.dram_tensor("out", [B_CORE, L, L], _F32, kind="ExternalOutput")

    with tile.TileContext(nc) as tc:
        with (
            tc.tile_pool(name="hank", bufs=1) as hank_pool,
            tc.tile_pool(name="a0", bufs=2) as a0_pool,
            tc.tile_pool(name="a1", bufs=2) as a1_pool,
            tc.tile_pool(name="ps0", bufs=2, space="PSUM") as ps0_pool,
            tc.tile_pool(name="ps1", bufs=2, space="PSUM") as ps1_pool,
            tc.tile_pool(name="osb", bufs=4) as osb_pool,
        ):
            zct = hank_pool.tile([128, 128], _F32, tag="zct")
            nc.scalar.dma_start(out=zct[:], in_=zc[:])
            # PE warm-up: ~4us of junk matmuls while the first hank DMA is
            # in flight, so the HAM clock gate reaches 8/8 before real work.
            warm = hank_pool.tile([128, 512], _F16, tag="warm")
            nc.vector.memset(warm[:], 0.0)
            wps = ps0_pool.tile([128, 512], _F32, tag="warmps")
            for _ in range(10):
                nc.tensor.matmul(
                    wps[:], lhsT=warm[:, 0:128], rhs=warm[:],
                    start=True, stop=True,
                )
            hanks = []
            for b in range(B_CORE):
                hank = hank_pool.tile([128, T, 256], _F16, tag=f"hank{b}")
                nc.sync.dma_start(out=hank[:, 0:4, :], in_=zh[b, :, 0:4, :])
                nc.sync.dma_start(out=hank[:, 4:8, :], in_=zh[b, :, 4:8, :])
                hanks.append(hank)

            for b in range(B_CORE):
                hank = hanks[b]
                a0 = a0_pool.tile([128, T, 256], _F16)
                a1 = a1_pool.tile([128, T, 128], _F16)
                for t in range(T):
                    # a0[p,t,l] = z[p] * z[p+l]      (DVE)
                    nc.vector.tensor_scalar_mul(
                        a0[:, t, :], hank[:, t, :], zct[:, 16 * b + t:16 * b + t + 1]
                    )
                    # a1[p,t,l] = z[128+p] * z[128+p+l]  (split DVE/ACT)
                    if t % 2 == 0:
                        nc.vector.tensor_scalar_mul(
                            a1[:, t, :], hank[:, t, 128:256],
                            zct[:, 16 * b + 8 + t:16 * b + 8 + t + 1],
                        )
                    else:
                        nc.scalar.activation(
                            a1[:, t, :], hank[:, t, 128:256], _COPY,
                            scale=zct[:, 16 * b + 8 + t:16 * b + 8 + t + 1],
                        )

                # Row-tile m=0: full l2 range; chunk1 feeds only l2 < 128.
                ps0 = ps0_pool.tile([128, 256], _F32)
                for t in range(T):
                    nc.tensor.matmul(
                        ps0[:], lhsT=a0[:, t, 0:128], rhs=hank[:, t, 0:256],
                        start=(t == 0), stop=False,
                    )
                for t in range(T):
                    nc.tensor.matmul(
                        ps0[:, 0:128], lhsT=a1[:, t, :],
                        rhs=hank[:, t, 128:256],
                        start=False, stop=(t == T - 1),
                    )
                # Row-tile m=1: only l2 in [128,256); lower-left quadrant is
                # the transpose of m=0's right half (host fills it).
                ps1 = ps1_pool.tile([128, 128], _F32)
                for t in range(T):
                    nc.tensor.matmul(
                        ps1[:], lhsT=a0[:, t, 128:256],
                        rhs=hank[:, t, 128:256],
                        start=(t == 0), stop=(t == T - 1),
                    )

                osb0 = osb_pool.tile([128, 256], _F32, tag="osb0")
                nc.scalar.activation(osb0[:], ps0[:], _COPY)
                nc.sync.dma_start(out=out[b, 0:128, :], in_=osb0[:])
                osb1 = osb_pool.tile([128, 128], _F32, tag="osb1")
                nc.scalar.activation(osb1[:], ps1[:], _COPY)
                nc.sync.dma_start(out=out[b, 128:256, 128:256], in_=osb1[:])
    nc.finalize()
    return nc


_NC_CACHE = None


def get_nc():
    global _NC_CACHE
    if _NC_CACHE is None:
        _NC_CACHE = _build_nc()
    return _NC_CACHE


def prepare_in_maps(target: np.ndarray):
    """Host prep: reversed/prescaled z, Hankel-window gather, fp16 cast."""
    target = np.ascontiguousarray(np.asarray(target, dtype=np.float32))
    assert target.shape == (B, T, L), target.shape
    s = np.float32((L * T) ** (-1.0 / 3.0))
    z = np.zeros((B, T, ZLEN), np.float32)
    z[:, :, :L] = target[:, :, ::-1] * s
    # win[b, t, p, j] = z[b, t, p+j]
    win = np.lib.stride_tricks.sliding_window_view(z, 256, axis=2)[:, :, :128, :]
    zh = np.ascontiguousarray(win.transpose(0, 2, 1, 3).astype(np.float16))
    # zc[p, 16b+8c+t] = z[b, t, 128c + p], zero-padded to 128 cols
    zcf = np.stack([z[:, :, 0:128], z[:, :, 128:256]], axis=2)  # [B, T, 2, 128]
    zcf = zcf.transpose(3, 0, 2, 1).reshape(128, B, 16)          # [128, B, 2*T]
    in_maps = [
        {
            "zh": np.ascontiguousarray(zh[i * B_CORE:(i + 1) * B_CORE]),
            "zc": np.concatenate(
                [zcf[:, i * B_CORE:(i + 1) * B_CORE].reshape(128, 64),
                 np.zeros((128, 64), np.float32)], axis=1),
        }
        for i in range(N_CORES)
    ]

    def assemble(results):
        source = np.concatenate(
            [results[i]["out"] for i in range(N_CORES)], axis=0
        )
        # lower-left quadrant by symmetry
        source[:, 128:256, 0:128] = source[:, 0:128, 128:256].transpose(0, 2, 1)
        return source[:, None, :, :]

    return in_maps, assemble


def kernel(target: np.ndarray) -> tuple[np.ndarray, np.ndarray]:
    target = np.ascontiguousarray(np.asarray(target, dtype=np.float32))
    in_maps, assemble = prepare_in_maps(target)
    res = run_bass_kernel_spmd(get_nc(), in_maps, list(range(N_CORES)))
    return assemble(res.results), target
